# revision 21
# baseline (speedup 1.0000x reference)
"""CTC loss on 8 trn2 NeuronCores.

Design:
- Batch B=64 split 8/core for the memory-bound part: each core streams its
  own slice of predicts through ACT exp(+accum) for the log_softmax
  denominators, which factor out of the CTC DP entirely
  (loss = -(ln L + renorms - sum_t ln denom_t)).
- predicts and the chain factors are cast to bf16 on the host: the rel-err
  budget is 2e-2 and bf16 rounding lands ~1e-4 here, while halving the
  27MB/core HBM stream. That makes ACT's exp throughput (1 elem/cycle/
  lane @1.2GHz) the bound, so the stream is organized to keep ACT
  saturated: one EXP per piece, a single Exp table set (the raw per-(t,
  piece) sums go out via DMA and the host does log().sum()), and the
  first piece is a half-sample so ACT starts early.
- Every stream piece has a DEDICATED SBUF buffer (bf16 makes them fit):
  all stream DMAs are dispatched up front on the Sync queue with no
  write-after-read hazards, so no dispatch ever blocks the FIFO and the
  DMA engines run free. Out-DMAs are dispatched last.
- The T=128-step CTC DP runs in linear space with periodic renorm
  (every 16 steps; factors are exp(bf16 logits) <= ~90 so f32 headroom
  is ample). The serial chain is split in half across core pairs: even
  cores run the FORWARD chain for the pair's 16 samples, odd cores the
  BACKWARD (suffix) chain, both as the *identical* SPMD program — the
  direction lives entirely in host-prepared data (s-axis reversed for
  backward, transition masks baked in as -1e30 logits, E_127 absorbed
  into the backward init). Both chains are 63 steps of 3 fused DVE ops +
  1 final multiply; cores combine L = sum_s alpha_63[s] * gamma_63[s] on
  host. gcat is DMA'd first so the chain starts as early as possible.
"""

from contextlib import ExitStack

import numpy as np
import ml_dtypes

import concourse.bacc as bacc
import concourse.tile as tile
import concourse.mybir as mybir
from concourse.ap import AP
from concourse.bass_utils import run_bass_kernel_spmd

B, T, C, L = 64, 128, 6625, 25
S = 2 * L + 1  # 51
M = 8          # cores
BS = B // M    # own samples per core (denominator stream)
PS = 2 * BS    # pair samples per core (DP chain)
NSTEP = 63
NSLOT = 64     # 63 steps + final-multiply slot
RENORM = 16
NREN = 4       # 3 in-chain renorms + 1 pre-final
GW = NSLOT * 2 * S  # gcat width (6528)
# sample 0 in small leading pieces (ACT starts early), samples 1-7 whole
PLAN = (
    [(0, 0, 1657), (0, 1657, 1656), (0, 3313, 3312)]
    + [(b, 0, C) for b in range(1, BS)]
)
NDEN = len(PLAN)      # 10 accumulator columns
DEN_SPLIT = 6         # cols [0,6) DMA'd out mid-stream, rest at the end
F32 = mybir.dt.float32
BF16 = mybir.dt.bfloat16

_cached = {}


def _dup_free(ap, n):
    """AP reading the free range of `ap` n times: [.., (0,n), (step,cnt)]."""
    dims = [list(d) for d in ap.ap]
    return AP(ap.tensor, ap.offset, dims[:-1] + [[0, n]] + [dims[-1]])


def _strided2(ap, gap, n):
    """AP over `ap`'s tensor writing two n-wide blocks `gap` apart."""
    dims = [list(d) for d in ap.ap]
    return AP(ap.tensor, ap.offset, dims[:-1] + [[gap, 2], [1, n]])


def _build():
    if "nc" in _cached:
        return _cached["nc"]
    nc = bacc.Bacc(
        "TRN2", target_bir_lowering=False, debug=False, num_devices=M
    )
    x = nc.dram_tensor("x", [BS, T, C], BF16, kind="ExternalInput").ap()
    gcat = nc.dram_tensor("gcat", [PS, GW], F32, kind="ExternalInput").ap()
    yinit = nc.dram_tensor("yinit", [PS, S], F32, kind="ExternalInput").ap()
    xpk = nc.dram_tensor("xpk", [PS, S + NREN], F32, kind="ExternalOutput").ap()
    dsum = nc.dram_tensor("dsum", [T, NDEN], F32, kind="ExternalOutput").ap()

    EXP = mybir.ActivationFunctionType.Exp
    MULT = mybir.AluOpType.mult

    with tile.TileContext(nc) as tc, ExitStack() as ctx:
        cpool = ctx.enter_context(tc.tile_pool(name="consts", bufs=1))

        # --- chain inputs first (chain start gates the tail), then the
        # stream pieces, each into its own dedicated buffer. gcat arrives
        # already exponentiated (host precomputes exp) so the chain and
        # the stream EXPs both start as soon as data lands. ---
        et = cpool.tile([PS, GW], F32)
        GH = GW // 2
        nc.sync.dma_start(et[:, 0:GH], gcat[:, 0:GH])
        nc.sync.dma_start(et[:, GH:], gcat[:, GH:])
        y_sb = cpool.tile([PS, S], F32)
        nc.sync.dma_start(y_sb[:], yinit)
        xts = []
        for i, (b, c0, cw) in enumerate(PLAN):
            xt = cpool.tile([128, cw], BF16, name=f"xt{i}")
            nc.sync.dma_start(xt[:], x[b, :, c0 : c0 + cw])
            xts.append(xt)

        # --- denominator stream EXPs (ACT-bound; outputs never read) ---
        junk = cpool.tile([128, C], BF16)
        den = cpool.tile([128, NDEN], F32)
        for i, (b, c0, cw) in enumerate(PLAN):
            nc.scalar.activation(
                junk[:, 0:cw], xts[i][:], EXP, accum_out=den[:, i : i + 1]
            )

        # --- DP chain: 63 steps of 3 fused DVE ops ---
        # wcat layout: [pad2 | w(51) | pad2 | wc(51)] = 106 cols
        wcat = cpool.tile([PS, 2 * S + 4], F32)
        u_t = cpool.tile([PS, S], F32)
        xpack = cpool.tile([PS, S + NREN], F32)  # [X(51) | ys(4)]
        inv = cpool.tile([PS, 1], F32)
        nc.vector.memset(wcat[:], 0.0)

        w_view = _strided2(wcat[:, 2 : 2 + S], 53, S)
        ys = xpack[:, S : S + NREN]
        jren = 0
        pending = False
        for k in range(1, NSTEP + 1):
            off = (k - 1) * 2 * S
            ek = et[:, off : off + 2 * S].rearrange(
                "p (two s) -> p two s", two=2
            )
            if pending:
                nc.vector.scalar_tensor_tensor(
                    w_view, _dup_free(y_sb[:], 2), inv[:], ek, MULT, MULT
                )
                pending = False
            else:
                nc.vector.tensor_mul(w_view, _dup_free(y_sb[:], 2), ek)
            nc.vector.tensor_add(u_t[:], wcat[:, 2 : 2 + S], wcat[:, 1 : 1 + S])
            nc.vector.tensor_add(y_sb[:], u_t[:], wcat[:, S + 2 : 2 * S + 2])
            if k % RENORM == 0:
                nc.vector.reduce_max(ys[:, jren : jren + 1], y_sb[:],
                                     axis=mybir.AxisListType.X)
                nc.vector.reciprocal(inv[:], ys[:, jren : jren + 1])
                pending = True
                jren += 1

        # final multiply (slot 64 A-half: fwd E_63 / bwd ones) + renorm
        nc.vector.reduce_max(ys[:, jren : jren + 1], y_sb[:],
                             axis=mybir.AxisListType.X)
        nc.vector.reciprocal(inv[:], ys[:, jren : jren + 1])
        jren += 1
        assert jren == NREN
        foff = NSTEP * 2 * S
        efin = et[:, foff : foff + S]
        nc.vector.scalar_tensor_tensor(
            xpack[:, 0:S], y_sb[:], inv[:], efin, MULT, MULT
        )

        # out-DMAs last on the Sync queue so their waits block nothing
        nc.sync.dma_start(dsum[:, 0:DEN_SPLIT], den[:, 0:DEN_SPLIT])
        nc.sync.dma_start(dsum[:, DEN_SPLIT:], den[:, DEN_SPLIT:])
        nc.sync.dma_start(xpk, xpack[:])

    nc.compile()
    _cached["nc"] = nc
    return nc


def _host_prep(predicts, labels, label_lengths):
    predicts = np.ascontiguousarray(np.asarray(predicts, dtype=np.float32))
    labels = np.asarray(labels).astype(np.int64)
    lens = np.asarray(label_lengths).astype(np.int64)

    ext = np.zeros((B, S), np.int64)
    ext[:, 1::2] = labels
    ext_sm2 = np.zeros((B, S), np.int64)
    ext_sm2[:, 2:] = ext[:, :-2]
    skip = ((ext != 0) & (ext != ext_sm2)).astype(np.float32)  # m[s]

    g = np.take_along_axis(predicts, ext[:, None, :], axis=2)  # [B,T,S] f32
    se = (2 * lens).astype(np.int64)
    for b in range(B):
        g[b, :, se[b] + 1 :] = -1e30  # s>2*len never feeds back

    endm = np.zeros((B, S), np.float32)
    endm[np.arange(B), se] = 1.0
    endm[np.arange(B), se - 1] = 1.0

    NEG = np.float32(-1e30)
    bf = ml_dtypes.bfloat16
    in_maps = []
    for m in range(M):
        p = m // 2
        sl = slice(16 * p, 16 * p + PS)       # pair samples
        gp, skp, enp = g[sl], skip[sl], endm[sl]
        gc = np.full((PS, NSLOT, 2, S), NEG, np.float32)
        yi = np.zeros((PS, S), np.float32)
        if m % 2 == 0:
            # forward: step k consumes E_{k-1}; A=g[k-1,s]; C=g[k-1,s'] if m[s'+2]
            for k in range(1, NSTEP + 1):
                gc[:, k - 1, 0, :] = gp[:, k - 1, :]
                cm = np.full((PS, S), NEG, np.float32)
                cm[:, : S - 2] = np.where(skp[:, 2:] > 0, gp[:, k - 1, : S - 2], NEG)
                gc[:, k - 1, 1, :] = cm
            gc[:, NSTEP, 0, :] = gp[:, NSTEP, :]  # final-mul slot: E_63
            yi[:, 0] = 1.0
            yi[:, 1] = 1.0
        else:
            # backward, s-reversed; init absorbs E_127; steps consume E_126..E_64
            gr = gp[:, :, ::-1]               # \hat g
            mr = skp[:, ::-1]                 # \hat m
            for k in range(1, NSTEP + 1):
                t = T - 2 - k                 # 125 .. 63; consumes E_{t+1}
                gc[:, k - 1, 0, :] = gr[:, t + 1, :]
                gc[:, k - 1, 1, :] = np.where(mr > 0, gr[:, t + 1, :], NEG)
            gc[:, NSTEP, 0, :] = 0.0          # final-mul slot: ones
            w = np.exp(gp[:, T - 1, :]) * enp
            wm = skp * w
            gm = w.copy()
            gm[:, : S - 1] += w[:, 1:]
            gm[:, : S - 2] += wm[:, 2:]
            yi[:] = gm[:, ::-1]
        in_maps.append({
            "x": np.ascontiguousarray(
                predicts[m * BS : (m + 1) * BS].astype(bf)
            ),
            "gcat": np.ascontiguousarray(np.exp(gc.reshape(PS, GW))),
            "yinit": yi,
        })
    return in_maps


def _run(in_maps, trace=False):
    nc = _build()
    res = run_bass_kernel_spmd(nc, in_maps, list(range(M)), trace=trace)
    losses = np.zeros(B, np.float32)
    for p in range(M // 2):
        re_, ro_ = res.results[2 * p], res.results[2 * p + 1]
        xe, xo = re_["xpk"][:, 0:S], ro_["xpk"][:, 0:S]
        yse, yso = re_["xpk"][:, S:], ro_["xpk"][:, S:]
        lv = (xe * xo[:, ::-1]).sum(axis=1, dtype=np.float32)
        tot = (np.log(lv) + np.log(yse).sum(1, dtype=np.float32)
               + np.log(yso).sum(1, dtype=np.float32))
        for half, r in ((0, re_), (1, ro_)):
            dnp = r["dsum"]  # [T, NDEN] raw chunk sums of exp
            dfull = np.zeros((T, BS), np.float32)
            for i, (b, _, _) in enumerate(PLAN):
                dfull[:, b] += dnp[:, i]
            dln = np.log(dfull).sum(axis=0, dtype=np.float32)  # [BS]
            losses[16 * p + 8 * half : 16 * p + 8 * half + BS] = (
                dln - tot[8 * half : 8 * half + BS]
            )
    losses = np.where(losses < 1e29, losses, 0.0).astype(np.float32)
    out = np.asarray(losses.mean(), dtype=np.float32)
    return out, res


def kernel(predicts, labels, label_lengths):
    in_maps = _host_prep(predicts, labels, label_lengths)
    out, _ = _run(in_maps, trace=False)
    return out


def kernel_traced(predicts, labels, label_lengths):
    in_maps = _host_prep(predicts, labels, label_lengths)
    return _run(in_maps, trace=True)


# revision 23
# speedup vs baseline: 1.0165x; 1.0165x over previous
"""CTC loss on 8 trn2 NeuronCores.

Design:
- Batch B=64 split 8/core for the memory-bound part: each core streams its
  own slice of predicts through ACT exp(+accum) for the log_softmax
  denominators, which factor out of the CTC DP entirely
  (loss = -(ln L + renorms - sum_t ln denom_t)).
- predicts and the chain factors are cast to bf16 on the host: the rel-err
  budget is 2e-2 and bf16 rounding lands ~1e-4 here, while halving the
  27MB/core HBM stream. That makes ACT's exp throughput (1 elem/cycle/
  lane @1.2GHz) the bound, so the stream is organized to keep ACT
  saturated: one EXP per piece, a single Exp table set (the raw per-(t,
  piece) sums go out via DMA and the host does log().sum()), and the
  first piece is a half-sample so ACT starts early.
- Every stream piece has a DEDICATED SBUF buffer (bf16 makes them fit):
  all stream DMAs are dispatched up front on the Sync queue with no
  write-after-read hazards, so no dispatch ever blocks the FIFO and the
  DMA engines run free. Out-DMAs are dispatched last.
- The T=128-step CTC DP runs in linear space with periodic renorm
  (every 16 steps; factors are exp(bf16 logits) <= ~90 so f32 headroom
  is ample). The serial chain is split in half across core pairs: even
  cores run the FORWARD chain for the pair's 16 samples, odd cores the
  BACKWARD (suffix) chain, both as the *identical* SPMD program — the
  direction lives entirely in host-prepared data (s-axis reversed for
  backward, transition masks baked in as -1e30 logits, E_127 absorbed
  into the backward init). Both chains are 63 steps of 3 fused DVE ops +
  1 final multiply; cores combine L = sum_s alpha_63[s] * gamma_63[s] on
  host. gcat is DMA'd first so the chain starts as early as possible.
"""

from contextlib import ExitStack

import numpy as np
import ml_dtypes

import concourse.bacc as bacc
import concourse.tile as tile
import concourse.mybir as mybir
from concourse.ap import AP
from concourse.bass_utils import run_bass_kernel_spmd

B, T, C, L = 64, 128, 6625, 25
S = 2 * L + 1  # 51
M = 8          # cores
BS = B // M    # own samples per core (denominator stream)
PS = 2 * BS    # pair samples per core (DP chain)
NSTEP = 63
NSLOT = 64     # 63 steps + final-multiply slot
RENORM = 16
NREN = 4       # 3 in-chain renorms + 1 pre-final
GW = NSLOT * 2 * S  # gcat width (6528)
# sample 0 as halves (ACT starts early), samples 1-7 whole
PLAN = (
    [(0, 0, 3313), (0, 3313, 3312)]
    + [(b, 0, C) for b in range(1, BS)]
)
NDEN = len(PLAN)      # 9 accumulator columns
DEN_SPLIT = 5         # cols [0,5) DMA'd out mid-stream, rest at the end
F32 = mybir.dt.float32
BF16 = mybir.dt.bfloat16

_cached = {}


def _dup_free(ap, n):
    """AP reading the free range of `ap` n times: [.., (0,n), (step,cnt)]."""
    dims = [list(d) for d in ap.ap]
    return AP(ap.tensor, ap.offset, dims[:-1] + [[0, n]] + [dims[-1]])


def _strided2(ap, gap, n):
    """AP over `ap`'s tensor writing two n-wide blocks `gap` apart."""
    dims = [list(d) for d in ap.ap]
    return AP(ap.tensor, ap.offset, dims[:-1] + [[gap, 2], [1, n]])


def _build():
    if "nc" in _cached:
        return _cached["nc"]
    nc = bacc.Bacc(
        "TRN2", target_bir_lowering=False, debug=False, num_devices=M
    )
    x = nc.dram_tensor("x", [BS, T, C], BF16, kind="ExternalInput").ap()
    gcat = nc.dram_tensor("gcat", [PS, GW], F32, kind="ExternalInput").ap()
    yinit = nc.dram_tensor("yinit", [PS, S], F32, kind="ExternalInput").ap()
    xpk = nc.dram_tensor("xpk", [PS, S + NREN], F32, kind="ExternalOutput").ap()
    dsum = nc.dram_tensor("dsum", [T, NDEN], F32, kind="ExternalOutput").ap()

    EXP = mybir.ActivationFunctionType.Exp
    MULT = mybir.AluOpType.mult

    with tile.TileContext(nc) as tc, ExitStack() as ctx:
        cpool = ctx.enter_context(tc.tile_pool(name="consts", bufs=1))

        # --- chain inputs first (chain start gates the tail), then the
        # stream pieces, each into its own dedicated buffer. gcat arrives
        # already exponentiated (host precomputes exp) so the chain and
        # the stream EXPs both start as soon as data lands. ---
        et = cpool.tile([PS, GW], F32)
        nc.sync.dma_start(et[:], gcat)
        y_sb = cpool.tile([PS, S], F32)
        nc.sync.dma_start(y_sb[:], yinit)
        xts = []
        for i, (b, c0, cw) in enumerate(PLAN):
            xt = cpool.tile([128, cw], BF16, name=f"xt{i}")
            nc.sync.dma_start(xt[:], x[b, :, c0 : c0 + cw])
            xts.append(xt)

        # --- denominator stream EXPs (ACT-bound; outputs never read) ---
        junk = cpool.tile([128, C], BF16)
        den = cpool.tile([128, NDEN], F32)
        for i, (b, c0, cw) in enumerate(PLAN):
            nc.scalar.activation(
                junk[:, 0:cw], xts[i][:], EXP, accum_out=den[:, i : i + 1]
            )

        # --- DP chain: 63 steps of 3 fused DVE ops ---
        # wcat layout: [pad2 | w(51) | pad2 | wc(51)] = 106 cols
        wcat = cpool.tile([PS, 2 * S + 4], F32)
        u_t = cpool.tile([PS, S], F32)
        xpack = cpool.tile([PS, S + NREN], F32)  # [X(51) | ys(4)]
        inv = cpool.tile([PS, 1], F32)
        nc.vector.memset(wcat[:], 0.0)

        w_view = _strided2(wcat[:, 2 : 2 + S], 53, S)
        ys = xpack[:, S : S + NREN]
        jren = 0
        pending = False
        for k in range(1, NSTEP + 1):
            off = (k - 1) * 2 * S
            ek = et[:, off : off + 2 * S].rearrange(
                "p (two s) -> p two s", two=2
            )
            if pending:
                nc.vector.scalar_tensor_tensor(
                    w_view, _dup_free(y_sb[:], 2), inv[:], ek, MULT, MULT
                )
                pending = False
            else:
                nc.vector.tensor_mul(w_view, _dup_free(y_sb[:], 2), ek)
            nc.vector.tensor_add(u_t[:], wcat[:, 2 : 2 + S], wcat[:, 1 : 1 + S])
            nc.vector.tensor_add(y_sb[:], u_t[:], wcat[:, S + 2 : 2 * S + 2])
            if k % RENORM == 0:
                nc.vector.reduce_max(ys[:, jren : jren + 1], y_sb[:],
                                     axis=mybir.AxisListType.X)
                nc.vector.reciprocal(inv[:], ys[:, jren : jren + 1])
                pending = True
                jren += 1

        # final multiply (slot 64 A-half: fwd E_63 / bwd ones) + renorm
        nc.vector.reduce_max(ys[:, jren : jren + 1], y_sb[:],
                             axis=mybir.AxisListType.X)
        nc.vector.reciprocal(inv[:], ys[:, jren : jren + 1])
        jren += 1
        assert jren == NREN
        foff = NSTEP * 2 * S
        efin = et[:, foff : foff + S]
        nc.vector.scalar_tensor_tensor(
            xpack[:, 0:S], y_sb[:], inv[:], efin, MULT, MULT
        )

        # out-DMAs last on the Sync queue so their waits block nothing
        nc.sync.dma_start(dsum[:, 0:DEN_SPLIT], den[:, 0:DEN_SPLIT])
        nc.sync.dma_start(dsum[:, DEN_SPLIT:], den[:, DEN_SPLIT:])
        nc.sync.dma_start(xpk, xpack[:])

    nc.compile()
    _cached["nc"] = nc
    return nc


def _host_prep(predicts, labels, label_lengths):
    predicts = np.ascontiguousarray(np.asarray(predicts, dtype=np.float32))
    labels = np.asarray(labels).astype(np.int64)
    lens = np.asarray(label_lengths).astype(np.int64)

    ext = np.zeros((B, S), np.int64)
    ext[:, 1::2] = labels
    ext_sm2 = np.zeros((B, S), np.int64)
    ext_sm2[:, 2:] = ext[:, :-2]
    skip = ((ext != 0) & (ext != ext_sm2)).astype(np.float32)  # m[s]

    g = np.take_along_axis(predicts, ext[:, None, :], axis=2)  # [B,T,S] f32
    se = (2 * lens).astype(np.int64)
    for b in range(B):
        g[b, :, se[b] + 1 :] = -1e30  # s>2*len never feeds back

    endm = np.zeros((B, S), np.float32)
    endm[np.arange(B), se] = 1.0
    endm[np.arange(B), se - 1] = 1.0

    NEG = np.float32(-1e30)
    bf = ml_dtypes.bfloat16
    in_maps = []
    for m in range(M):
        p = m // 2
        sl = slice(16 * p, 16 * p + PS)       # pair samples
        gp, skp, enp = g[sl], skip[sl], endm[sl]
        gc = np.full((PS, NSLOT, 2, S), NEG, np.float32)
        yi = np.zeros((PS, S), np.float32)
        if m % 2 == 0:
            # forward: step k consumes E_{k-1}; A=g[k-1,s]; C=g[k-1,s'] if m[s'+2]
            for k in range(1, NSTEP + 1):
                gc[:, k - 1, 0, :] = gp[:, k - 1, :]
                cm = np.full((PS, S), NEG, np.float32)
                cm[:, : S - 2] = np.where(skp[:, 2:] > 0, gp[:, k - 1, : S - 2], NEG)
                gc[:, k - 1, 1, :] = cm
            gc[:, NSTEP, 0, :] = gp[:, NSTEP, :]  # final-mul slot: E_63
            yi[:, 0] = 1.0
            yi[:, 1] = 1.0
        else:
            # backward, s-reversed; init absorbs E_127; steps consume E_126..E_64
            gr = gp[:, :, ::-1]               # \hat g
            mr = skp[:, ::-1]                 # \hat m
            for k in range(1, NSTEP + 1):
                t = T - 2 - k                 # 125 .. 63; consumes E_{t+1}
                gc[:, k - 1, 0, :] = gr[:, t + 1, :]
                gc[:, k - 1, 1, :] = np.where(mr > 0, gr[:, t + 1, :], NEG)
            gc[:, NSTEP, 0, :] = 0.0          # final-mul slot: ones
            w = np.exp(gp[:, T - 1, :]) * enp
            wm = skp * w
            gm = w.copy()
            gm[:, : S - 1] += w[:, 1:]
            gm[:, : S - 2] += wm[:, 2:]
            yi[:] = gm[:, ::-1]
        in_maps.append({
            "x": np.ascontiguousarray(
                predicts[m * BS : (m + 1) * BS].astype(bf)
            ),
            "gcat": np.ascontiguousarray(np.exp(gc.reshape(PS, GW))),
            "yinit": yi,
        })
    return in_maps


def _run(in_maps, trace=False):
    nc = _build()
    res = run_bass_kernel_spmd(nc, in_maps, list(range(M)), trace=trace)
    losses = np.zeros(B, np.float32)
    for p in range(M // 2):
        re_, ro_ = res.results[2 * p], res.results[2 * p + 1]
        xe, xo = re_["xpk"][:, 0:S], ro_["xpk"][:, 0:S]
        yse, yso = re_["xpk"][:, S:], ro_["xpk"][:, S:]
        lv = (xe * xo[:, ::-1]).sum(axis=1, dtype=np.float32)
        tot = (np.log(lv) + np.log(yse).sum(1, dtype=np.float32)
               + np.log(yso).sum(1, dtype=np.float32))
        for half, r in ((0, re_), (1, ro_)):
            dnp = r["dsum"]  # [T, NDEN] raw chunk sums of exp
            dfull = np.zeros((T, BS), np.float32)
            for i, (b, _, _) in enumerate(PLAN):
                dfull[:, b] += dnp[:, i]
            dln = np.log(dfull).sum(axis=0, dtype=np.float32)  # [BS]
            losses[16 * p + 8 * half : 16 * p + 8 * half + BS] = (
                dln - tot[8 * half : 8 * half + BS]
            )
    losses = np.where(losses < 1e29, losses, 0.0).astype(np.float32)
    out = np.asarray(losses.mean(), dtype=np.float32)
    return out, res


def kernel(predicts, labels, label_lengths):
    in_maps = _host_prep(predicts, labels, label_lengths)
    out, _ = _run(in_maps, trace=False)
    return out


def kernel_traced(predicts, labels, label_lengths):
    in_maps = _host_prep(predicts, labels, label_lengths)
    return _run(in_maps, trace=True)
